# revision 1
# baseline (speedup 1.0000x reference)
"""GAT (2-layer, 4-head) Bass kernel for Trainium2, data-parallel over 8 NeuronCores.

Math (per sample b, per attention instance with weights W, a = [a1; a2]):
    Wh = h @ W                      [N, F]
    s  = Wh @ a1   (per-dst-node i score part)
    t  = Wh @ a2   (per-src-node j score part)
    e[i,j]   = leaky_relu(s[i] + t[j], 0.2)
    att      = softmax_j(where(adj[i,j] > 0, e, -9e15))
    out[i]   = sum_j att[i,j] * Wh[j]

Kernel layout choices:
  - All N x N score/attention tiles live as [j(part), i(free)] "transposed"
    tiles: exp's affine pre-add fuses s (free-broadcast via a PE matmul
    against a column-replicated W@a1) with t (per-partition bias), and the
    final contraction over j runs on the PE with the small [Wh | ones]
    block as the stationary operand streaming the whole pT row -- one
    weight load per j-tile instead of one per 128x128 chunk.
  - exp(lrelu(z)) is exact via max(exp(z), exp(0.2 z)) (two ACT exps +
    DVE max) on NACT j-tiles per instance, and via fp16 z-space lrelu on
    DVE (z, 0.2z, max) + one ACT exp on the rest, balancing ACT vs DVE.
  - Masking is one batched DVE tensor_tensor mult per instance with the
    host-pre-transposed bf16 0/1 mask (equivalent to the -9e15 additive
    mask: every row has >= 1 edge at this density, and softmax
    normalization cancels the missing max-subtraction; |z| <~ 15 is safe
    in fp32 exp).
  - Softmax row-sums ride along as the ones column of the stationary
    [Wh | 1] block -> row 64 of the [65, N] PSUM output. Normalization:
    reciprocal_approx_fast on that row (~4e-6 rel err), PE-broadcast of
    the reciprocal row to [64, N], one TT multiply -- which also serves
    as the PSUM->SBUF copy into h_cat^T (layer 1) / the elu input
    (layer 2).
"""

import os
import sys

import numpy as np

if not os.path.isdir(os.path.join(os.path.dirname(os.path.abspath(__file__)), "concourse")):
    for _p in ("/opt/trn_rl_repo", os.path.expanduser("~/.axon_site/_ro/trn_rl_repo")):
        if os.path.isdir(_p) and _p not in sys.path:
            sys.path.append(_p)

import ml_dtypes  # noqa: E402

import concourse.bacc as bacc  # noqa: E402
import concourse.tile as tile  # noqa: E402
from concourse import mybir  # noqa: E402
from concourse.bass_utils import run_bass_kernel_spmd  # noqa: E402

BF16 = ml_dtypes.bfloat16

B, N, FIN, FH, H, FOUT = 16, 1024, 256, 64, 4, 64
NCORES = 8
SPC = B // NCORES  # samples per core
KT = FIN // 128    # k tiles (2)
JT = N // 128      # j tiles (8)
ALPHA = 0.2

# j-tiles [0, NACT) take the 2-exp ACT path; the rest build lrelu on DVE.
NACT = 3
USE_APPROX_RECIP = True

F32 = mybir.dt.float32
F16 = mybir.dt.float16
BF = mybir.dt.bfloat16
AF = mybir.ActivationFunctionType
OP = mybir.AluOpType
AX = mybir.AxisListType


def _register_leaky_op():
    """One-op fused leaky_relu with per-partition bias:
    out = max(in0 + s0[p], 0.2 * (in0 + s0[p])). Registered at import into
    the process-local custom-DVE op table (baked per-NEFF at compile)."""
    import concourse.dve_ops as dve_ops
    from concourse.dve_spec import C0, C1, Spec, Src0, lower, maxx
    from concourse.dve_uop import DveOpSpec

    name = "LEAKY_BIAS_ANT"
    for op in dve_ops.OPS:
        if op.name == name:
            return op
    zz = Src0 + C0
    spec = Spec(
        body=maxx(zz, zz * C1),
        reference=lambda in0, in1, s0, s1, imm2: np.maximum(
            (in0 + s0), (in0 + s0) * s1
        ).astype(np.float32),
    )
    row = max(dve_ops._SUB_OPCODE_FOR_NAME.values()) + 1
    assert row < 0x20
    dve_ops._SUB_OPCODE_FOR_NAME[name] = row
    shas = {}
    for ver in ("v3", "v4"):
        o = DveOpSpec(name=name, opcode=row, uops=lower(spec, ver=ver), rd1_en=False)
        shas[ver] = o.sha(ver)
    op = dve_ops.DveOp(name, spec, subdim=False, uops_sha=shas)
    dve_ops.OPS.append(op)
    dve_ops.CUSTOM_DVE_SPECS[name] = spec
    return op


LEAKY_OP = _register_leaky_op()


def _gat_instance(nc, pools, maskT_sb, inst, emit_norm):
    """One attention instance (a head of L1, or L2).

    inst:
      rep(kt)  -> AP [128,128] bf16  column-replicated W@a1 (S matmul lhsT)
      rhs(kt)  -> AP [128,1024] bf16 x^T / h_cat^T k-tile
      wh(jt)   -> AP [128,65]  bf16  [Wh block | ones col] (att matmul lhsT)
      tcol(jt) -> AP [128,1] f32     t bias column
      t02(jt)  -> AP [128,1] f32     0.2*t column
      ones64   -> AP [1,64] f32      for the reciprocal broadcast matmul
    emit_norm(ot_ps, rbc_ps): consume the unnormalized [65,N] output + the
      [64,N] broadcast reciprocal row (both PSUM).
    """
    work, psA, psO = pools["work"], pools["psA"], pools["psO"]
    workbig = pools["workbig"]

    # S_bcast[p, i] = s[i] for all p.
    sb_ps = psA.tile([128, N], F32, tag="big")
    for kt in range(KT):
        for ih in range(2):
            nc.tensor.matmul(
                sb_ps[:, ih * 512 : (ih + 1) * 512],
                inst["rep"](kt),
                inst["rhs"](kt)[:, ih * 512 : (ih + 1) * 512],
                start=(kt == 0),
                stop=(kt == KT - 1),
            )
    sb16 = work.tile([128, N], F16, tag="sb16")
    nc.scalar.copy(sb16, sb_ps)

    pT = workbig.tile([128, JT, N], BF, tag="pt")
    for jt in range(JT):
        p = work.tile([128, N], BF, tag="p")
        if jt < NACT:
            # exp(lrelu(z)) == max(exp(z), exp(0.2 z)), z = S_bcast + t.
            e1 = work.tile([128, N], BF, tag="e1")
            nc.scalar.activation(e1, sb_ps, AF.Exp, bias=inst["tcol"](jt), scale=1.0)
            e2 = work.tile([128, N], BF, tag="e2")
            nc.scalar.activation(e2, sb_ps, AF.Exp, bias=inst["t02"](jt), scale=ALPHA)
            nc.vector.tensor_tensor(p, e1, e2, OP.max)
            nc.vector.tensor_tensor(pT[:, jt, :], p, maskT_sb[:, jt, :], OP.mult)
        else:
            # fused z-space lrelu on DVE (one custom op), then one ACT exp.
            ell = work.tile([128, N], F16, tag="ell")
            nc.vector._custom_dve(
                LEAKY_OP, out=ell, in0=sb16, s0=inst["tcol"](jt), s1=ALPHA
            )
            nc.scalar.activation(p, ell, AF.Exp)
            nc.vector.tensor_tensor(pT[:, jt, :], p, maskT_sb[:, jt, :], OP.mult)

    # O^T[f, i] (+ rowsum row 64) accumulated over j-tiles.
    ot_ps = psO.tile([FH + 1, N], F32, tag="ot")
    for jt in range(JT):
        for ih in range(2):
            nc.tensor.matmul(
                ot_ps[:, ih * 512 : (ih + 1) * 512],
                inst["wh"](jt),
                pT[:, jt, ih * 512 : (ih + 1) * 512],
                start=(jt == 0),
                stop=(jt == JT - 1),
            )

    # Deferred tail: 1/rowsum + PE broadcast + normalize. Returned as a
    # closure so the caller can emit it AFTER the next instance's main body
    # -- otherwise the in-order PE queue head-of-line blocks on the
    # reciprocal and every engine idles for the whole tail chain.
    def finish():
        # O^T rows to SBUF (ACT, bf16) in parallel with the reciprocal chain.
        ot_sb = work.tile([FH, N], BF, tag="otsb")
        nc.scalar.copy(ot_sb, ot_ps[0:FH, :])
        # custom-DVE ops read garbage from PSUM on HW -- stage the row in SBUF
        rs = work.tile([1, N], F32, tag="rs")
        nc.vector.tensor_copy(rs, ot_ps[FH : FH + 1, :])
        rb = work.tile([1, N], F32, tag="rb")
        if USE_APPROX_RECIP:
            nc.vector.reciprocal_approx_fast(out=rb, in_=rs)
        else:
            nc.vector.reciprocal(out=rb, in_=rs)
        rb_bf = work.tile([1, N], BF, tag="rbbf")
        nc.scalar.copy(rb_bf, rb)
        rbc_ps = psA.tile([FH, N], F32, tag="big")
        for ih in range(2):
            nc.tensor.matmul(
                rbc_ps[:, ih * 512 : (ih + 1) * 512],
                inst["ones64"],
                rb_bf[:, ih * 512 : (ih + 1) * 512],
                start=True,
                stop=True,
            )
        emit_norm(ot_sb, rbc_ps)

    return finish


def _build_nc():
    nc = bacc.Bacc()

    xT_d = nc.declare_dram_parameter("xT", [SPC, KT, 128, N], BF, isOutput=False)
    maskT_d = nc.declare_dram_parameter("maskT", [SPC, JT, 128, N], BF, isOutput=False)
    wbig1_d = nc.declare_dram_parameter("wbig1", [KT, 128, H * 65 + H], BF, isOutput=False)
    warep1_d = nc.declare_dram_parameter("warep1", [KT, 128, H * 128], BF, isOutput=False)
    wbig2_d = nc.declare_dram_parameter("wbig2", [KT, 128, 66], BF, isOutput=False)
    warep2_d = nc.declare_dram_parameter("warep2", [KT, 128, 128], BF, isOutput=False)
    out_d = nc.declare_dram_parameter("out", [SPC, FOUT], F32, isOutput=True)

    with tile.TileContext(nc) as tc:
        with (
            tc.tile_pool(name="const", bufs=1) as constp,
            tc.tile_pool(name="samp", bufs=2) as samp,
            tc.tile_pool(name="workbig", bufs=2) as workbig,
            tc.tile_pool(name="work", bufs=3) as work,
            tc.tile_pool(name="tail", bufs=1) as tail,
            tc.tile_pool(name="psA", bufs=2, space="PSUM") as psA,
            tc.tile_pool(name="psO", bufs=2, space="PSUM") as psO,
        ):
            pools = {"work": work, "workbig": workbig, "psA": psA, "psO": psO}

            wbig1_sb = constp.tile([128, KT, H * 65 + H], BF)
            warep1_sb = constp.tile([128, KT, H * 128], BF)
            wbig2_sb = constp.tile([128, KT, 66], BF)
            warep2_sb = constp.tile([128, KT, 128], BF)
            for kt in range(KT):
                nc.sync.dma_start(out=wbig1_sb[:, kt, :], in_=wbig1_d[kt])
                nc.sync.dma_start(out=warep1_sb[:, kt, :], in_=warep1_d[kt])
                nc.sync.dma_start(out=wbig2_sb[:, kt, :], in_=wbig2_d[kt])
                nc.sync.dma_start(out=warep2_sb[:, kt, :], in_=warep2_d[kt])
            ones64_sb = constp.tile([1, FH], BF)
            nc.vector.memset(ones64_sb, 1.0)

            pending_sample_tail = None
            for s in range(SPC):
                xT_sb = samp.tile([128, KT, N], BF, tag="xt")
                for kt in range(KT):
                    nc.sync.dma_start(out=xT_sb[:, kt, :], in_=xT_d[s, kt])
                maskT_sb = samp.tile([128, JT, N], BF, tag="mask")
                for jt in range(JT):
                    nc.sync.dma_start(out=maskT_sb[:, jt, :], in_=maskT_d[s, jt])

                # ---- L1 Wh for all 4 heads (+ t columns) ----
                whsb1 = samp.tile([128, JT, H * 65], BF, tag="whsb1")
                tc1 = samp.tile([128, JT, H], F32, tag="tc1")
                t02_1 = samp.tile([128, JT, H], F32, tag="t02_1")
                for jt in range(JT):
                    wm_ps = psA.tile([128, H * 65 + H], F32, tag="big")
                    for kt in range(KT):
                        nc.tensor.matmul(
                            wm_ps,
                            xT_sb[:, kt, jt * 128 : (jt + 1) * 128],
                            wbig1_sb[:, kt, :],
                            start=(kt == 0),
                            stop=(kt == KT - 1),
                        )
                    nc.scalar.copy(whsb1[:, jt, :], wm_ps[:, 0 : H * 65])
                    nc.vector.memset(whsb1[:, jt, FH : H * 65 : 65], 1.0)
                    nc.vector.tensor_copy(tc1[:, jt, :], wm_ps[:, H * 65 : H * 65 + H])
                    nc.vector.tensor_scalar(
                        t02_1[:, jt, :], wm_ps[:, H * 65 : H * 65 + H], ALPHA, None, OP.mult
                    )

                if pending_sample_tail is not None:
                    pending_sample_tail()
                    pending_sample_tail = None

                # ---- L1 attention, 4 heads -> h_cat^T ----
                hcatT = samp.tile([128, KT, N], BF, tag="hcat")

                pending = None
                for h in range(H):
                    def emit_l1(ot_ps, rbc_ps, h=h):
                        dst = hcatT[(h % 2) * 64 : (h % 2) * 64 + 64, h // 2, :]
                        nc.vector.tensor_tensor(dst, ot_ps[0:FH, :], rbc_ps, OP.mult)

                    fin = _gat_instance(
                        nc,
                        pools,
                        maskT_sb,
                        {
                            "rep": lambda kt, h=h: warep1_sb[:, kt, h * 128 : (h + 1) * 128],
                            "rhs": lambda kt: xT_sb[:, kt, :],
                            "wh": lambda jt, h=h: whsb1[:, jt, h * 65 : (h + 1) * 65],
                            "tcol": lambda jt, h=h: tc1[:, jt, h : h + 1],
                            "t02": lambda jt, h=h: t02_1[:, jt, h : h + 1],
                            "ones64": ones64_sb,
                        },
                        emit_l1,
                    )
                    if pending is not None:
                        pending()
                    pending = fin
                pending()

                # ---- L2 Wh ----
                whsb2 = samp.tile([128, JT, 65], BF, tag="whsb2")
                tc2 = samp.tile([128, JT, 1], F32, tag="tc2")
                t02_2 = samp.tile([128, JT, 1], F32, tag="t02_2")
                for jt in range(JT):
                    wm_ps = psA.tile([128, 66], F32, tag="big")
                    for kt in range(KT):
                        nc.tensor.matmul(
                            wm_ps,
                            hcatT[:, kt, jt * 128 : (jt + 1) * 128],
                            wbig2_sb[:, kt, :],
                            start=(kt == 0),
                            stop=(kt == KT - 1),
                        )
                    nc.vector.tensor_copy(whsb2[:, jt, 0:FOUT], wm_ps[:, 0:FOUT])
                    nc.vector.memset(whsb2[:, jt, FOUT : FOUT + 1], 1.0)
                    nc.vector.tensor_copy(tc2[:, jt, :], wm_ps[:, 65:66])
                    nc.vector.tensor_scalar(t02_2[:, jt, :], wm_ps[:, 65:66], ALPHA, None, OP.mult)

                # ---- L2 attention + elu + mean ----
                o2n = tail.tile([FH, N], F32, tag="o2n")

                def emit_l2(ot_ps, rbc_ps, o2n=o2n):
                    nc.vector.tensor_tensor(o2n, ot_ps[0:FH, :], rbc_ps, OP.mult)

                fin2 = _gat_instance(
                    nc,
                    pools,
                    maskT_sb,
                    {
                        "rep": lambda kt: warep2_sb[:, kt, :],
                        "rhs": lambda kt: hcatT[:, kt, :],
                        "wh": lambda jt: whsb2[:, jt, :],
                        "tcol": lambda jt: tc2[:, jt, :],
                        "t02": lambda jt: t02_2[:, jt, :],
                        "ones64": ones64_sb,
                    },
                    emit_l2,
                )
                def sample_tail(s=s, fin2=fin2, o2n=o2n):
                    fin2()
                    # elu(x) = relu(x) + min(exp(x)-1, 0); mean over nodes =
                    # free-axis reduce; DMA the [64,1] column to out[s].
                    ex = tail.tile([FH, N], F32, tag="ex")
                    nc.scalar.activation(ex, o2n, AF.Exp)
                    bmax = tail.tile([FH, N], F32, tag="bmax")
                    nc.scalar.activation(bmax, o2n, AF.Relu)
                    cmin = tail.tile([FH, N], F32, tag="cmin")
                    nc.vector.tensor_scalar(cmin, ex, -1.0, 0.0, OP.add, OP.min)
                    eluv = tail.tile([FH, N], F32, tag="eluv")
                    nc.vector.tensor_tensor(eluv, bmax, cmin, OP.add)
                    red = tail.tile([FH, 1], F32, tag="red")
                    nc.vector.tensor_reduce(red, eluv, axis=AX.X, op=OP.add)
                    outc = tail.tile([FH, 1], F32, tag="outc")
                    nc.vector.tensor_scalar(outc, red, 1.0 / N, None, OP.mult)
                    nc.sync.dma_start(
                        out=out_d[s].rearrange("(f a) -> f a", a=1), in_=outc
                    )

                pending_sample_tail = sample_tail

            pending_sample_tail()

    nc.finalize()
    return nc


_NC_CACHE = None


def _prep_host(x, adj, W_heads, a_heads, W_out, a_out):
    xT = np.ascontiguousarray(np.asarray(x, np.float32).transpose(0, 2, 1)).astype(BF16)
    xT = xT.reshape(B, KT, 128, N)
    maskT = (np.asarray(adj) > 0).transpose(0, 2, 1).astype(BF16)  # [B, j, i]
    maskT = np.ascontiguousarray(maskT).reshape(B, JT, 128, N)

    W_heads = np.asarray(W_heads, np.float32)
    a_heads = np.asarray(a_heads, np.float32)
    W_out = np.asarray(W_out, np.float32)
    a_out = np.asarray(a_out, np.float32)

    wbig1 = np.zeros((FIN, H * 65 + H), dtype=np.float32)
    warep1 = np.zeros((FIN, H * 128), dtype=np.float32)
    for h in range(H):
        Wh_ = W_heads[h]
        wbig1[:, h * 65 : h * 65 + FH] = Wh_
        wbig1[:, H * 65 + h] = Wh_ @ a_heads[h, FH:, 0]
        warep1[:, h * 128 : (h + 1) * 128] = (Wh_ @ a_heads[h, :FH, 0])[:, None]
    wbig2 = np.zeros((FIN, 66), dtype=np.float32)
    wbig2[:, 0:FOUT] = W_out
    wbig2[:, 65] = W_out @ a_out[FOUT:, 0]
    warep2 = np.repeat((W_out @ a_out[:FOUT, 0])[:, None], 128, axis=1)

    shared = {
        "wbig1": wbig1.astype(BF16).reshape(KT, 128, H * 65 + H),
        "warep1": warep1.astype(BF16).reshape(KT, 128, H * 128),
        "wbig2": wbig2.astype(BF16).reshape(KT, 128, 66),
        "warep2": warep2.astype(BF16).reshape(KT, 128, 128),
    }
    in_maps = []
    for c in range(NCORES):
        sl = slice(c * SPC, (c + 1) * SPC)
        m = {"xT": np.ascontiguousarray(xT[sl]), "maskT": np.ascontiguousarray(maskT[sl])}
        m.update(shared)
        in_maps.append(m)
    return in_maps


def kernel(x, adj, W_heads, a_heads, W_out, a_out, _trace=False):
    global _NC_CACHE
    if _NC_CACHE is None:
        _NC_CACHE = _build_nc()
    nc = _NC_CACHE
    in_maps = _prep_host(x, adj, W_heads, a_heads, W_out, a_out)
    res = run_bass_kernel_spmd(nc, in_maps, core_ids=list(range(NCORES)), trace=_trace)
    out = np.concatenate([res.results[c]["out"] for c in range(NCORES)], axis=0)
    if _trace:
        kernel._last_results = res
    return out.astype(np.float32)

